# revision 1
# baseline (speedup 1.0000x reference)
"""Bidirectional cross-attention Trainium2 kernel (8 NeuronCores, SPMD).

Sharding: 2 heads per core (16 heads / 8 cores); both batches on every core.
Host pre-transposes x/context to [D, B*N] bf16, slices per-head weight columns.
Device: projections -> two symmetric attention passes (row softmax of sim and
of sim^T, via unnormalized exp + fused denominator row) -> AllToAll to
sequence-shard the merged heads -> output projections -> per-core slices.
"""

import numpy as np
import ml_dtypes

BF16 = ml_dtypes.bfloat16

# problem constants (hardcoded per contract)
B = 2
N = 2048
D = 1024
HEADS = 16
DIM_HEAD = 64
SCALE = DIM_HEAD ** -0.5

N_CORES = 8
HD = 128            # per-core head dims (2 heads x 64)
A = B * N           # 4096 flattened tokens
SL = N // N_CORES   # 256 per-batch output slice per core
KC = D // 128       # 8 contraction chunks for projections
AC_W = 512          # projection output chunk width
N_AT = 2048 // 128  # 16 partition tiles per batch in attention


def build_nc(reps=1, use_cc=True):
    import concourse.bacc as bacc
    import concourse.tile as tile
    from concourse import mybir
    from concourse.masks import make_identity

    fp32 = mybir.dt.float32
    bf16 = mybir.dt.bfloat16
    EXP = mybir.ActivationFunctionType.Exp

    nc = bacc.Bacc("TRN2", target_bir_lowering=False, num_devices=N_CORES)

    # ---- I/O ----
    xT = nc.dram_tensor("xT", [D, A], bf16, kind="ExternalInput")
    cT = nc.dram_tensor("cT", [D, A], bf16, kind="ExternalInput")
    wqk = nc.dram_tensor("wqk", [D, HD], bf16, kind="ExternalInput")
    wv = nc.dram_tensor("wv", [D, HD], bf16, kind="ExternalInput")
    wcqk = nc.dram_tensor("wcqk", [D, HD], bf16, kind="ExternalInput")
    wcv = nc.dram_tensor("wcv", [D, HD], bf16, kind="ExternalInput")
    wout = nc.dram_tensor("wout", [D, D], bf16, kind="ExternalInput")
    wcout = nc.dram_tensor("wcout", [D, D], bf16, kind="ExternalInput")
    out_sl = nc.dram_tensor("out_sl", [2 * SL, D], fp32, kind="ExternalOutput")
    ctx_sl = nc.dram_tensor("ctx_sl", [2 * SL, D], fp32, kind="ExternalOutput")

    # collective bounce buffers, one per direction x batch (shard-major)
    cc_in_o0 = nc.dram_tensor("cc_in_o0", [N_CORES * HD, SL], bf16)
    cc_in_o1 = nc.dram_tensor("cc_in_o1", [N_CORES * HD, SL], bf16)
    cc_out_o0 = nc.dram_tensor("cc_out_o0", [N_CORES * HD, SL], bf16)
    cc_out_o1 = nc.dram_tensor("cc_out_o1", [N_CORES * HD, SL], bf16)
    cc_in_c0 = nc.dram_tensor("cc_in_c0", [N_CORES * HD, SL], bf16)
    cc_in_c1 = nc.dram_tensor("cc_in_c1", [N_CORES * HD, SL], bf16)
    cc_out_c0 = nc.dram_tensor("cc_out_c0", [N_CORES * HD, SL], bf16)
    cc_out_c1 = nc.dram_tensor("cc_out_c1", [N_CORES * HD, SL], bf16)

    with tile.TileContext(nc) as tc:
        with tc.tile_pool(name="singles", bufs=1) as singles:
            # ---- long-lived SBUF tensors ----
            wqk_sb = singles.tile([128, KC, HD], bf16)
            wv_sb = singles.tile([128, KC, HD], bf16)
            wcqk_sb = singles.tile([128, KC, HD], bf16)
            wcv_sb = singles.tile([128, KC, HD], bf16)
            for w_dram, w_sb in ((wqk, wqk_sb), (wv, wv_sb), (wcqk, wcqk_sb), (wcv, wcv_sb)):
                wv_ = w_dram.ap().rearrange("(k p) m -> p k m", p=128)
                nc.sync.dma_start(out=w_sb[:, 0:1, :], in_=wv_[:, 0:1, :])
                nc.sync.dma_start(out=w_sb[:, 1:KC, :], in_=wv_[:, 1:KC, :])

            wout_sb = singles.tile([128, KC, D], bf16)
            wcout_sb = singles.tile([128, KC, D], bf16)

            qkT_sb = singles.tile([128, A], bf16)     # [hd, a]
            cqkT_sb = singles.tile([128, A], bf16)    # [hd, c]
            # per-head natural-layout values with fused ones column: blocks of 65
            v0_sb = singles.tile([128, A // 128, 65], bf16)
            v1_sb = singles.tile([128, A // 128, 65], bf16)
            cv0_sb = singles.tile([128, A // 128, 65], bf16)
            cv1_sb = singles.tile([128, A // 128, 65], bf16)
            for t in (v0_sb, v1_sb, cv0_sb, cv1_sb):
                nc.vector.memset(t, 1.0)

            ident = singles.tile([128, 128], bf16)
            make_identity(nc, ident)

            ones_col = singles.tile([65, 64], bf16)
            nc.vector.memset(ones_col, 1.0)

            # per-head unnormalized attention outputs [d, pos]
            oa_h0 = singles.tile([64, A], bf16)   # dir1: out
            oa_h1 = singles.tile([64, A], bf16)
            ca_h0 = singles.tile([64, A], bf16)   # dir2: ctx_out
            ca_h1 = singles.tile([64, A], bf16)
            oa_h = [oa_h0, oa_h1]
            ca_h = [ca_h0, ca_h1]

            for _rep in range(reps):
                # ================= P1: projections =================
                with (
                    tc.tile_pool(name="p1sb", bufs=3) as p1sb,
                    tc.tile_pool(name="p1scr", bufs=1) as p1scr,
                ):
                    vT_sb = p1scr.tile([128, A], bf16)    # [hd, a] pre-transpose scratch
                    cvT_sb = p1scr.tile([128, A], bf16)
                    n_ac = A // AC_W
                    ps1cm = tc.tile_pool(name="ps1", bufs=8, space="PSUM")
                    ps1 = ps1cm.__enter__()
                    for ac in range(n_ac):
                        x_t = p1sb.tile([128, KC, AC_W], bf16, tag="xin")
                        c_t = p1sb.tile([128, KC, AC_W], bf16, tag="cin")
                        nsplit = KC if ac == 0 else 2
                        step = KC // nsplit
                        for si in range(nsplit):
                            ks = slice(si * step, (si + 1) * step)
                            nc.sync.dma_start(
                                out=x_t[:, ks, :],
                                in_=xT.ap().rearrange("(k p) a -> p k a", p=128)[:, ks, ac * AC_W:(ac + 1) * AC_W],
                            )
                            nc.sync.dma_start(
                                out=c_t[:, ks, :],
                                in_=cT.ap().rearrange("(k p) a -> p k a", p=128)[:, ks, ac * AC_W:(ac + 1) * AC_W],
                            )
                        qk_ps = ps1.tile([128, AC_W], fp32, tag="p1")
                        v_ps = ps1.tile([128, AC_W], fp32, tag="p1")
                        cqk_ps = ps1.tile([128, AC_W], fp32, tag="p1")
                        cv_ps = ps1.tile([128, AC_W], fp32, tag="p1")
                        for kc in range(KC):
                            st = kc == 0
                            sp = kc == KC - 1
                            nc.tensor.matmul(qk_ps, wqk_sb[:, kc, :], x_t[:, kc, :], start=st, stop=sp)
                            nc.tensor.matmul(v_ps, wv_sb[:, kc, :], x_t[:, kc, :], start=st, stop=sp)
                            nc.tensor.matmul(cqk_ps, wcqk_sb[:, kc, :], c_t[:, kc, :], start=st, stop=sp)
                            nc.tensor.matmul(cv_ps, wcv_sb[:, kc, :], c_t[:, kc, :], start=st, stop=sp)
                        sl_ = slice(ac * AC_W, (ac + 1) * AC_W)
                        nc.vector.tensor_copy(qkT_sb[:, sl_], qk_ps)
                        nc.vector.tensor_copy(cqkT_sb[:, sl_], cqk_ps)
                        nc.scalar.copy(vT_sb[:, sl_], v_ps)
                        nc.scalar.copy(cvT_sb[:, sl_], cv_ps)

                    ps1cm.__exit__(None, None, None)
                    # ---- P1b: transpose v/cv to natural layout, split heads ----
                    ps1tcm = tc.tile_pool(name="ps1t", bufs=4, space="PSUM")
                    ps1t = ps1tcm.__enter__()
                    for src_sb, d0, d1 in ((vT_sb, v0_sb, v1_sb), (cvT_sb, cv0_sb, cv1_sb)):
                        for ac in range(n_ac):
                            tr_ps = ps1t.tile([128, AC_W], bf16, tag="tr")
                            for i in range(AC_W // 128):
                                blk = ac * AC_W + i * 128
                                nc.tensor.transpose(
                                    tr_ps[:, i * 128:(i + 1) * 128],
                                    src_sb[:, blk:blk + 128],
                                    ident,
                                )
                            trv = tr_ps.rearrange("p (i m) -> p i m", m=128)
                            nb = AC_W // 128
                            a0 = ac * nb
                            nc.vector.tensor_copy(d0[:, a0:a0 + nb, 0:64], trv[:, :, 0:64])
                            nc.vector.tensor_copy(d1[:, a0:a0 + nb, 0:64], trv[:, :, 64:128])

                    ps1tcm.__exit__(None, None, None)
                # ================= P2: attention =================
                with (
                    tc.tile_pool(name="e_pool", bufs=8) as e_pool,
                    tc.tile_pool(name="norm", bufs=3) as norm_pool,
                    tc.tile_pool(name="ps2", bufs=2, space="PSUM") as ps2,
                ):
                    def attn_pass(b, statT, movT, vo0, vo1, dst0, dst1):
                        # sim tiles [stat-pos(128), mov-pos], exp, PV with fused
                        # denominator row, normalize, write dst [64, mov-pos].
                        base = b * 2048
                        for cw in range(2):  # 1024-wide mov window
                            wbase = base + cw * 1024
                            pv0 = ps2.tile([65, 1024], fp32, tag="pv")
                            pv1 = ps2.tile([65, 1024], fp32, tag="pv")
                            for at in range(N_AT):
                                a0 = base + at * 128
                                for cs in range(2):  # 512-wide sub-window
                                    m0 = wbase + cs * 512
                                    s = ps2.tile([128, 1024], fp32, tag="sim")
                                    nc.tensor.matmul(
                                        s[:, 0:512],
                                        statT[0:64, a0:a0 + 128],
                                        movT[0:64, m0:m0 + 512],
                                        start=True, stop=True,
                                    )
                                    nc.tensor.matmul(
                                        s[:, 512:1024],
                                        statT[64:128, a0:a0 + 128],
                                        movT[64:128, m0:m0 + 512],
                                        start=True, stop=True,
                                        tile_position=(64, 0),
                                    )
                                    e = e_pool.tile([128, 1024], bf16, tag="e")
                                    nc.scalar.activation(e, s, EXP, scale=SCALE)
                                    gat = b * N_AT + at
                                    st = at == 0
                                    sp = at == N_AT - 1
                                    nc.tensor.matmul(
                                        pv0[:, cs * 512:(cs + 1) * 512],
                                        vo0[:, gat, :], e[:, 0:512],
                                        start=st, stop=sp, skip_group_check=True,
                                    )
                                    nc.tensor.matmul(
                                        pv1[:, cs * 512:(cs + 1) * 512],
                                        vo1[:, gat, :], e[:, 512:1024],
                                        start=st, stop=sp, skip_group_check=True,
                                    )
                            # normalize: dst = pv[0:64] * bcast(1 / pv[64]).
                            # pv is copied to SBUF first; the dead pv PSUM rows
                            # then host the broadcast so no sim slot is taken.
                            for pv, dst in ((pv0, dst0), (pv1, dst1)):
                                pvsb = norm_pool.tile([65, 1024], fp32, tag="pvsb")
                                nc.vector.tensor_copy(pvsb, pv)
                                nc.vector.reciprocal(pvsb[64:65, :], pvsb[64:65, :])
                                recb = norm_pool.tile([65, 1024], bf16, tag="recb")
                                nc.vector.tensor_copy(recb[64:65, :], pvsb[64:65, :])
                                for g in range(2):
                                    nc.tensor.matmul(
                                        pv[0:64, g * 512:(g + 1) * 512],
                                        ones_col[64:65, :],
                                        recb[64:65, g * 512:(g + 1) * 512],
                                        start=True, stop=True, skip_group_check=True,
                                    )
                                nc.vector.tensor_mul(
                                    dst[:, wbase:wbase + 1024], pvsb[0:64, :], pv[0:64, :]
                                )

                    def bounce_and_a2a(srcs, b, cc_in, cc_out):
                        ccv = cc_in.ap().rearrange("(r p) j -> r p j", p=HD)
                        for r in range(N_CORES):
                            for h, src_t in enumerate(srcs):
                                nc.sync.dma_start(
                                    out=ccv[r, h * 64:(h + 1) * 64],
                                    in_=src_t[:, b * 2048 + r * SL:b * 2048 + (r + 1) * SL],
                                )
                        if use_cc:
                            nc.gpsimd.collective_compute(
                                "AllToAll", mybir.AluOpType.bypass,
                                replica_groups=[list(range(N_CORES))],
                                ins=[cc_in.ap().opt()], outs=[cc_out.ap().opt()],
                            )
                        else:
                            nc.sync.dma_start(out=cc_out.ap(), in_=cc_in.ap())

                    # pass A (dir2: context_out):  stat=qk(a), mov=cqk(c), values=v
                    attn_pass(0, qkT_sb, cqkT_sb, v0_sb, v1_sb, ca_h[0], ca_h[1])
                    bounce_and_a2a(ca_h, 0, cc_in_c0, cc_out_c0)
                    attn_pass(1, qkT_sb, cqkT_sb, v0_sb, v1_sb, ca_h[0], ca_h[1])
                    bounce_and_a2a(ca_h, 1, cc_in_c1, cc_out_c1)
                    # pass B (dir1: out):  stat=cqk(c), mov=qk(a), values=cv
                    attn_pass(0, cqkT_sb, qkT_sb, cv0_sb, cv1_sb, oa_h[0], oa_h[1])
                    bounce_and_a2a(oa_h, 0, cc_in_o0, cc_out_o0)
                    nc.sync.dma_start(out=wcout_sb, in_=wcout.ap().rearrange("(k p) g -> p k g", p=128))
                    nc.sync.dma_start(out=wout_sb, in_=wout.ap().rearrange("(k p) g -> p k g", p=128))
                    attn_pass(1, cqkT_sb, qkT_sb, cv0_sb, cv1_sb, oa_h[0], oa_h[1])

                    # ============ P4: output projections (inside P2 pools) ============
                    with (
                        tc.tile_pool(name="s3in", bufs=2) as s3in,
                        tc.tile_pool(name="s3out", bufs=2) as s3out,
                    ):
                        def stage3(full, w_sb, dst, at2_range):
                            for at2 in at2_range:
                                ps3 = ps2.tile([128, D], fp32, tag="sim")
                                for kc in range(KC):
                                    for g in range(2):
                                        nc.tensor.matmul(
                                            ps3[:, g * 512:(g + 1) * 512],
                                            full[:, kc, at2 * 128:(at2 + 1) * 128],
                                            w_sb[:, kc, g * 512:(g + 1) * 512],
                                            start=(kc == 0), stop=(kc == KC - 1),
                                            skip_group_check=True,
                                        )
                                o_sb = s3out.tile([128, D], fp32, tag="o")
                                nc.vector.tensor_copy(o_sb, ps3)
                                nc.sync.dma_start(
                                    out=dst.ap()[at2 * 128:(at2 + 1) * 128, :], in_=o_sb
                                )

                        # issue everything that does NOT depend on the last A2A
                        # (ctx both halves, oa b0 half) BEFORE the last collective,
                        # so PE fills the collective's latency and no DMA-lane
                        # false-serialization gates it.
                        full_c = s3in.tile([128, KC, 2 * SL], bf16, tag="full")
                        for bi, cc in enumerate((cc_out_c0, cc_out_c1)):
                            nc.sync.dma_start(
                                out=full_c[:, :, bi * SL:(bi + 1) * SL],
                                in_=cc.ap().rearrange("(k p) a -> p k a", p=128),
                            )
                        # bounce DMAs for oa-b1 (collective issued after stage3 work)
                        ccv = cc_in_o1.ap().rearrange("(r p) j -> r p j", p=HD)
                        for r in range(N_CORES):
                            for h, src_t in enumerate(oa_h):
                                nc.sync.dma_start(
                                    out=ccv[r, h * 64:(h + 1) * 64],
                                    in_=src_t[:, 2048 + r * SL:2048 + (r + 1) * SL],
                                )
                        full_o = s3in.tile([128, KC, 2 * SL], bf16, tag="full")
                        nc.gpsimd.dma_start(
                            out=full_o[:, :, 0:SL],
                            in_=cc_out_o0.ap().rearrange("(k p) a -> p k a", p=128),
                        )
                        if use_cc:
                            nc.gpsimd.collective_compute(
                                "AllToAll", mybir.AluOpType.bypass,
                                replica_groups=[list(range(N_CORES))],
                                ins=[cc_in_o1.ap().opt()], outs=[cc_out_o1.ap().opt()],
                            )
                        else:
                            nc.gpsimd.dma_start(out=cc_out_o1.ap(), in_=cc_in_o1.ap())
                        stage3(full_c, wcout_sb, ctx_sl, range(4))
                        stage3(full_o, wout_sb, out_sl, range(2))
                        nc.gpsimd.dma_start(
                            out=full_o[:, :, SL:2 * SL],
                            in_=cc_out_o1.ap().rearrange("(k p) a -> p k a", p=128),
                        )
                        stage3(full_o, wout_sb, out_sl, range(2, 4))
    nc.compile()
    return nc


_NC_CACHE = {}


def _get_nc():
    if "nc" not in _NC_CACHE:
        _NC_CACHE["nc"] = build_nc()
    return _NC_CACHE["nc"]


def _run(in_maps, trace=False):
    from concourse.bass_utils import run_bass_kernel_spmd
    nc = _get_nc()
    return run_bass_kernel_spmd(nc, in_maps, core_ids=list(range(N_CORES)), trace=trace)


def prepare_in_maps(x, context, w_qk, w_v, w_cqk, w_cv, w_out, w_cout):
    x = np.asarray(x, dtype=np.float32)
    context = np.asarray(context, dtype=np.float32)
    xT = np.ascontiguousarray(x.reshape(A, D).T).astype(BF16)
    cT = np.ascontiguousarray(context.reshape(A, D).T).astype(BF16)
    wout_b = np.asarray(w_out, np.float32).astype(BF16)
    wcout_b = np.asarray(w_cout, np.float32).astype(BF16)
    in_maps = []
    for c in range(N_CORES):
        cs = slice(c * HD, (c + 1) * HD)
        in_maps.append({
            "xT": xT,
            "cT": cT,
            "wqk": np.ascontiguousarray(np.asarray(w_qk, np.float32)[:, cs]).astype(BF16),
            "wv": np.ascontiguousarray(np.asarray(w_v, np.float32)[:, cs]).astype(BF16),
            "wcqk": np.ascontiguousarray(np.asarray(w_cqk, np.float32)[:, cs]).astype(BF16),
            "wcv": np.ascontiguousarray(np.asarray(w_cv, np.float32)[:, cs]).astype(BF16),
            "wout": wout_b,
            "wcout": wcout_b,
        })
    return in_maps


def assemble(results):
    out = np.empty((B, N, D), np.float32)
    ctx = np.empty((B, N, D), np.float32)
    for c in range(N_CORES):
        o = results[c]["out_sl"].reshape(B, SL, D)
        k = results[c]["ctx_sl"].reshape(B, SL, D)
        out[:, c * SL:(c + 1) * SL, :] = o
        ctx[:, c * SL:(c + 1) * SL, :] = k
    return out, ctx


def kernel(x, context, w_qk, w_v, w_cqk, w_cv, w_out, w_cout):
    in_maps = prepare_in_maps(x, context, w_qk, w_v, w_cqk, w_cv, w_out, w_cout)
    res = _run(in_maps)
    return assemble(res.results)

